# revision 22
# baseline (speedup 1.0000x reference)
"""Trainium2 Bass kernel for nn_ExpAffineQuantizer (log2-domain fake quantization).

Full inputs -> shard rows across 8 NeuronCores -> raw-Bass SPMD kernel -> gather.

Math per 128-element group g of x:
    scale_g = clip(max(|sig(fu_g)*max(xg)|, |sig(fl_g)*min(xg)|), 1e-5, 1e4)
    out = sign(x) * scale_g * 2^clip(round(log2(|x|/scale_g)), -7, 0)

Fast path (equal factors, no exact-zero x, scale strictly inside the clip
range, sigmoid(factor) > 1/sqrt2): a hardware-looped pipeline whose NEFF size
is constant in `repeat` (the loop count is an immediate).  Per 128-row chunk,
8 DVE ops (4.5 element-touches; DVE measures 98.9% busy — the floor, since
every 2-input DVE op takes the SBUF port pair shared with GpSimd, so no
engine overlap is possible for this op mix):
    V1  mx = group absmax(x)                 (DVE reduce, 1x)
    A1  sg = sigmoid(fu)                     (ACT)
    V2  sc = sg*mx;  tb = sc/sqrt2;  u2 = reciprocal(tb)   (DVE HW divide)
    V6  y  = x*u2                            (DVE TT broadcast)
    V7  y &= 0x7F800000                      (DVE TS int, 2x) -> 2^floor(log2|y|)
    V8  y  = max(y, 2^-7) * sc               (DVE STT broadcast)
    V9  out= (bits(x) & sign) | bits(y)      (DVE STT int imm)
V9 relies on sigmoid > 1/sqrt2 so |x|*sqrt2/sc < 2 and the QMAX=0 clip is a
no-op (no upper clamp needed); _pick_config only selects this path then.

repeat>1 re-runs the same pass in-device via per-engine Fori loops; semaphore
waits use per-series registers advanced by constants each iteration, with
pre-seeded semaphores so first-iteration targets stay non-negative.  Output
DMAs issue from the ACT engine so the sync engine never stalls on end-of-pass
compute.  One phantom out-DMA (iteration 0, slot 0) writes garbage to the
last chunk's rows and is overwritten by the real output on the same HWDGE
ring (FIFO order).

Toolchain constraints (probed on this stack):
  - Tile/TileContext kernels don't compile; raw Bass with explicit semaphore
    edges is required (same-engine back-to-back dependent ops race).
  - Mixing bitwise and arith ALU ops in one instruction is rejected.
  - scalar_tensor_tensor with bitvec ops needs an int32-typed immediate
    (hand-built InstTensorScalarPtr; the python helper lowers imms as f32).
  - Pool (gpsimd) supports only float mult/add TensorTensor; no TS/STT, no
    max, no bitwise.  DVE 2-port TS perf modes lock Pool out of SBUF.
  - Two in-flight DMAs on one semaphore complete out of order; completion
    sems are parity-split per double-buffer slot.

Fallback configs (unequal factors, zeros in x, clip not provably dead) use
the original unrolled builder (slower, fully general).
"""
import functools
from contextlib import ExitStack

import numpy as np

import concourse.bass as bass
from concourse import mybir
from concourse.bass_utils import run_bass_kernel_spmd

F32 = mybir.dt.float32
I32 = mybir.dt.int32
AF = mybir.ActivationFunctionType
Alu = mybir.AluOpType

GROUP = 128
SQRT2 = float(np.sqrt(2.0))
EXP_MASK = 0x7F800000
SIGN_BIT = -0x80000000  # int32-encodable sign mask
ONE_BITS = 0x3F800000
SIGNLESS = 0x7FFFFFFF
TWO_M7 = float(2.0 ** -7)
CLIPMIN = 1e-05
CLIPMAX = 10000.0
N_CORES = 8

D1, D2 = 4096, 11008
SR = D1 // N_CORES


def act_reciprocal(nc, out_ap, in_ap):
    """ACT Reciprocal (bass blocks it by default; ~1.2e-5 rel err, fine as NR seed)."""
    return nc.scalar.add_instruction(
        mybir.InstActivation(
            name=nc.get_next_instruction_name(),
            func=AF.Reciprocal,
            ins=[
                nc.scalar.lower_ap(in_ap),
                mybir.ImmediateValue(dtype=F32, value=0.0),
                mybir.ImmediateValue(dtype=F32, value=1.0),
                mybir.ImmediateValue(dtype=F32, value=0.0),
            ],
            outs=[nc.scalar.lower_ap(out_ap)],
        )
    )


def stt_int(nc, eng, out, in0, scalar_int, in1, op0, op1):
    """scalar_tensor_tensor with an int32-typed immediate (bitvec op pairs)."""
    return eng.add_instruction(
        mybir.InstTensorScalarPtr(
            name=nc.get_next_instruction_name(),
            is_scalar_tensor_tensor=True,
            op0=op0,
            op1=op1,
            ins=[
                eng.lower_ap(in0),
                mybir.ImmediateValue(dtype=I32, value=scalar_int),
                eng.lower_ap(in1),
            ],
            outs=[eng.lower_ap(out)],
        )
    )


class _Series:
    """One monotone semaphore-wait series on one engine, tracked in a register.

    Wait targets are `iter * per_iter + c` with compile-time c.  The loop body
    is emitted once, so every delta must be iteration-invariant: the register
    is pre-seeded to the first target, in-body waits advance by c_k - c_{k-1},
    and end_iter() advances to the next iteration's first target."""

    def __init__(self, ctx, eng, sem, per_iter, name, first_c):
        self.eng, self.sem, self.per_iter = eng, sem, per_iter
        self.first_c = first_c
        self.reg = ctx.enter_context(eng.register(name))
        eng.reg_mov(self.reg, first_c)
        self.last = first_c

    def wait(self, c):
        d = c - self.last
        assert d >= 0, f"series {self.sem}: target {c} < last {self.last}"
        if d:
            self.eng.reg_add(self.reg, self.reg, d)
            self.last = c
        return self.eng.wait_ge(self.sem, self.reg)

    def end_iter(self):
        d = self.per_iter + self.first_c - self.last
        assert d >= 0
        if d:
            self.eng.reg_add(self.reg, self.reg, d)
        self.last = self.first_c


def _emit_sem_init(nc, sems):
    """Zero this kernel's semaphores at entry, then NRT-barrier all engines.

    Executions are not guaranteed fresh semaphores (NEFF re-execution in one
    process reuses the loaded program; nc.semaphore never clears).  The
    pseudo-barrier expands to RT-internal semaphores, so it is safe while the
    bass sems still hold garbage."""
    nums = sorted(s.num for s in sems)
    assert nums == list(range(nums[0], nums[0] + len(nums))), nums
    rng = range(nums[0], nums[-1] + 1)
    nc.gpsimd.dma_reset(rng)
    nc.gpsimd.sem_clear(rng)
    nc._nrt_pseudo_barrier()


def build_loop_kernel(sr=SR, d2=D2, cg=86, repeat=1):
    """Fast-path one-core program with an in-device loop over `repeat`."""
    d2g = d2 // GROUP
    assert d2g == cg, "loop kernel runs full-width chunks"
    assert sr % 256 == 0, "even chunk count needed for double buffering"
    rb = sr // 128            # chunks per pass
    ce = cg * GROUP           # chunk width in elements
    SVd, SAd = 8 * rb, rb     # per-iteration increments of s_v / s_a
    XDd = 8 * rb               # per-buffer per-iteration DMA sem increments
    SV_SEED, SA_SEED = 16, 1
    INV_SQRT2 = float(1.0 / np.sqrt(2.0))

    nc = bass.Bass()
    x = nc.dram_tensor("x", [sr, d2], F32, kind="ExternalInput")
    fu = nc.dram_tensor("fu", [sr, d2g], F32, kind="ExternalInput")
    out = nc.dram_tensor("out", [sr, d2], F32, kind="ExternalOutput")

    def rows(j):
        return slice((j % rb) * 128, (j % rb + 1) * 128)

    with ExitStack() as ctx:
        def sb2(name, shape, dt=F32):
            return [ctx.enter_context(nc.sbuf_tensor(f"{name}{k}", shape, dt))
                    for k in range(2)]
        xt = sb2("xt", [128, ce])
        yt = sb2("yt", [128, ce])
        ftu = sb2("ftu", [128, cg])
        sg = sb2("sg", [128, cg])
        mx = sb2("mx", [128, cg])
        sc = sb2("sc", [128, cg])
        ta = sb2("ta", [128, cg])
        tb = sb2("tb", [128, cg])
        u2 = sb2("u2", [128, cg])
        s_xd = [ctx.enter_context(nc.semaphore(f"s_xd{k}")) for k in range(2)]
        s_fd = [ctx.enter_context(nc.semaphore(f"s_fd{k}")) for k in range(2)]
        s_od = [ctx.enter_context(nc.semaphore(f"s_od{k}")) for k in range(2)]
        s_v = ctx.enter_context(nc.semaphore("s_v"))
        s_a = ctx.enter_context(nc.semaphore("s_a"))
        _emit_sem_init(nc, s_xd + s_fd + s_od + [s_v, s_a])
        block = ctx.enter_context(nc.Block())

        @block.sync
        def _(sync):
            with ExitStack() as ectx:
                sv = _Series(ectx, sync, s_v, SVd, "sy_sv", 8)
                sa = _Series(ectx, sync, s_a, SAd, "sy_sa", 0)
                with sync.Fori(0, repeat):
                    for j in range(rb):
                        b = j & 1
                        # xt[b] free when chunk j-2's V9 consumed it
                        sv.wait(8 * j + 8)
                        # ftu[b] free when chunk j-2's sigmoid consumed it
                        sa.wait(j)
                        sync.dma_start(xt[b][:], x[rows(j), :]).then_inc(
                            s_xd[b], 16)
                        sync.dma_start(ftu[b][:], fu[rows(j), :]).then_inc(
                            s_fd[b], 16)
                    sv.end_iter()
                    sa.end_iter()

        @block.scalar
        def _(scalar):
            with ExitStack() as ectx:
                scalar.sem_inc(s_a, SA_SEED)
                sv_a1 = _Series(ectx, scalar, s_v, SVd, "sc_sv_a1", 2)
                sv_out = _Series(ectx, scalar, s_v, SVd, "sc_sv_out", 16)
                fd = [_Series(ectx, scalar, s_fd[k], XDd, f"sc_fd{k}", 16)
                      for k in range(2)]
                with scalar.Fori(0, repeat):
                    for j in range(rb):
                        b = j & 1
                        # A1: sigmoid. sg[b] free when chunk j-2's V2 done.
                        sv_a1.wait(8 * j + 2)
                        fd[b].wait(16 * (j // 2) + 16)
                        nc.scalar.activation(
                            sg[b][:], ftu[b][:], AF.Sigmoid).then_inc(s_a, 1)
                        # A3: output chunk j-1 (j=0: prev iteration's last
                        # chunk; iteration 0 emits one phantom, overwritten
                        # by the real last-chunk output on the same ring).
                        pb = (j - 1) & 1
                        sv_out.wait(8 * j + 16)
                        scalar.dma_start(
                            out[rows(j - 1), :], yt[pb][:]).then_inc(
                            s_od[pb], 16)
                    sv_a1.end_iter()
                    sv_out.end_iter()
                    for k in range(2):
                        fd[k].end_iter()
                # epilogue: final chunk's output (target SVd*rep + 16)
                last = (rb - 1) & 1
                sv_out.wait(16)
                scalar.dma_start(out[rows(rb - 1), :], yt[last][:]).then_inc(
                    s_od[last], 16)

        @block.vector
        def _(vector):
            with ExitStack() as ectx:
                vector.sem_inc(s_v, SV_SEED)
                vc = _Series(ectx, vector, s_v, SVd, "v_chain", SV_SEED)
                va = _Series(ectx, vector, s_a, SAd, "v_sa", 2)
                xd = [_Series(ectx, vector, s_xd[k], XDd, f"v_xd{k}", 16)
                      for k in range(2)]
                od = [_Series(ectx, vector, s_od[k], XDd, f"v_od{k}", 16 * k)
                      for k in range(2)]
                with vector.Fori(0, repeat):
                    for j in range(rb):
                        b = j & 1
                        n0 = 8 * j  # global V-op count before this chunk
                        x3 = xt[b][:].rearrange("p (g e) -> p g e", g=cg)
                        y3 = yt[b][:].rearrange("p (g e) -> p g e", g=cg)
                        u2b = u2[b][:].unsqueeze(2).broadcast_to(
                            (128, cg, GROUP))
                        scb = sc[b][:].unsqueeze(2).broadcast_to(
                            (128, cg, GROUP))

                        # V1: group absmax
                        xd[b].wait(16 * (j // 2) + 16)
                        vc.wait(SV_SEED + n0)
                        nc.vector.tensor_reduce(
                            mx[b][:], x3, axis=mybir.AxisListType.X,
                            op=Alu.max, apply_absolute_value=True,
                        ).then_inc(s_v, 1)
                        # V2: sc = sg*mx
                        va.wait(j + 2)
                        vc.wait(SV_SEED + n0 + 1)
                        nc.vector.tensor_tensor(
                            sc[b][:], sg[b][:], mx[b][:], Alu.mult,
                        ).then_inc(s_v, 1)
                        # u2 = sqrt2/sc via DVE iterative divide
                        vc.wait(SV_SEED + n0 + 2)
                        nc.vector.tensor_scalar(
                            tb[b][:], sc[b][:], INV_SQRT2, None,
                            Alu.mult).then_inc(s_v, 1)
                        vc.wait(SV_SEED + n0 + 3)
                        nc.vector.reciprocal(
                            u2[b][:], tb[b][:]).then_inc(s_v, 1)
                        # V6: y = x*u2   (yt[b] free when its out-DMA done)
                        od[b].wait(16 * ((j + b) // 2))
                        vc.wait(SV_SEED + n0 + 4)
                        nc.vector.tensor_tensor(
                            y3, x3, u2b, Alu.mult,
                        ).then_inc(s_v, 1)
                        # V7: exponent-field extraction
                        vc.wait(SV_SEED + n0 + 5)
                        nc.vector.tensor_scalar(
                            yt[b][:].bitcast(I32), yt[b][:].bitcast(I32),
                            EXP_MASK, None, Alu.bitwise_and,
                        ).then_inc(s_v, 1)
                        # V8: QMIN clip + rescale
                        vc.wait(SV_SEED + n0 + 6)
                        nc.vector.scalar_tensor_tensor(
                            y3, y3, TWO_M7, scb, Alu.max, Alu.mult,
                        ).then_inc(s_v, 1)
                        # V9: reapply sign from x's bits
                        vc.wait(SV_SEED + n0 + 7)
                        stt_int(
                            nc, nc.vector,
                            yt[b][:].bitcast(I32), xt[b][:].bitcast(I32),
                            SIGN_BIT, yt[b][:].bitcast(I32),
                            Alu.bitwise_and, Alu.bitwise_or,
                        ).then_inc(s_v, 1)
                    vc.end_iter()
                    va.end_iter()
                    for k in range(2):
                        xd[k].end_iter()
                        od[k].end_iter()

    return nc


def build_loop_v2(sr=SR, d2=D2, repeat=1, pool=True):
    """Fast path v2: DVE+Pool+ACT op-split, half-width chunks, hw loop.

    Per slot (chunk = 128 rows x 5504 cols; prev chunk p processed skewed):
      ACT : abs(j) -> yt[b]; sigmoid(j); out-DMA of chunk p after V9(p)
      Pool: F(p): yt[p] *= sc (bcast)  |  C(j): yt[b] *= u2 (bcast)
      DVE : V7(p) exponent mask, TSmax(p) QMIN clip, reduce(j),
            sc=sg*mx, tb=sc/sqrt2, u2=recip(tb), V9(p) sign-OR
      sync: x half-chunk in; fu row-block in (even slots)
    All DVE TS ops use an odd-innermost 3D view so they stay in 1x mode and
    never grab the SBUF port pair Pool depends on.
    """
    d2g = d2 // GROUP
    assert d2g % 2 == 0 and sr % 256 == 0
    cw = d2g // 2            # groups per half-width chunk
    ce = cw * GROUP
    R = (sr // 128) * 2      # slots per pass
    assert R % 4 == 0
    O = 7 if pool else 9     # DVE s_v incs per slot
    SVd, SAd, SPd = O * R, 2 * R, 2 * R
    XDd, ODd, FDd = 4 * R, 8 * R, 4 * R
    SV_SEED, SA_SEED = 2 * O + 2, 4
    INV_SQRT2 = float(1.0 / np.sqrt(2.0))

    nc = bass.Bass()
    x = nc.dram_tensor("x", [sr, d2], F32, kind="ExternalInput")
    fu = nc.dram_tensor("fu", [sr, d2g], F32, kind="ExternalInput")
    out = nc.dram_tensor("out", [sr, d2], F32, kind="ExternalOutput")

    def xs(j):
        r, h = (j % R) // 2, (j % R) & 1
        return slice(r * 128, (r + 1) * 128), slice(h * ce, (h + 1) * ce)

    def g3(tile):
        return tile[:].rearrange("p (g e) -> p g e", g=cw)

    def odd3(ap2d):
        # [128, ce] -> [128, 128, cw]: innermost odd => DVE 1x mode
        return ap2d.rearrange("p (a b) -> p a b", b=cw)

    def bc(small):
        return small[:].unsqueeze(2).broadcast_to((128, cw, GROUP))

    with ExitStack() as ctx:
        def sbn(name, shape, n, dt=F32):
            return [ctx.enter_context(nc.sbuf_tensor(f"{name}{k}", shape, dt))
                    for k in range(n)]
        xt = sbn("xt", [128, ce], 4)
        yt = sbn("yt", [128, ce], 2)
        ftu = sbn("ftu", [128, d2g], 2)
        sg = sbn("sg", [128, cw], 2)
        mx = sbn("mx", [128, cw], 2)
        sc = sbn("sc", [128, cw], 2)
        tb = sbn("tb", [128, cw], 2)
        u2 = sbn("u2", [128, cw], 2)
        s_xd = [ctx.enter_context(nc.semaphore(f"s_xd{k}")) for k in range(4)]
        s_fd = [ctx.enter_context(nc.semaphore(f"s_fd{k}")) for k in range(2)]
        s_od = [ctx.enter_context(nc.semaphore(f"s_od{k}")) for k in range(2)]
        s_v = ctx.enter_context(nc.semaphore("s_v"))
        s_a = ctx.enter_context(nc.semaphore("s_a"))
        s_p = ctx.enter_context(nc.semaphore("s_p"))
        _emit_sem_init(nc, s_xd + s_fd + s_od + [s_v, s_a, s_p])
        block = ctx.enter_context(nc.Block())

        @block.sync
        def _(sync):
            with ExitStack() as ectx:
                sv = _Series(ectx, sync, s_v, SVd, "sy_sv", 2)
                sa = _Series(ectx, sync, s_a, SAd, "sy_sa", 0)
                with sync.Fori(0, repeat):
                    for j in range(R):
                        bx = j % 4
                        # xt[bx] free once V9(chunk j-4) read its sign bits
                        sv.wait(O * j + 2)
                        sync.dma_start(xt[bx][:], x[xs(j)]).then_inc(
                            s_xd[bx], 16)
                        if j % 2 == 0:
                            bf = (j // 2) & 1
                            # ftu[bf] free once sigmoid(chunk j-3) read it
                            sa.wait(2 * j)
                            rs = slice((j // 2) * 128, (j // 2 + 1) * 128)
                            sync.dma_start(ftu[bf][:], fu[rs, :]).then_inc(
                                s_fd[bf], 16)
                    sv.end_iter()
                    sa.end_iter()

        @block.scalar
        def _(scalar):
            with ExitStack() as ectx:
                scalar.sem_inc(s_a, SA_SEED)
                xd = [_Series(ectx, scalar, s_xd[k], XDd, f"sc_xd{k}", 16)
                      for k in range(4)]
                od = [_Series(ectx, scalar, s_od[k], ODd, f"sc_od{k}", 16 * k)
                      for k in range(2)]
                fd = [_Series(ectx, scalar, s_fd[k], FDd, f"sc_fd{k}", 16)
                      for k in range(2)]
                c_sg = 2 + (4 if pool else 5)      # V2 mark offset
                c_out = SV_SEED + O                # V9 mark offset
                svsg = _Series(ectx, scalar, s_v, SVd, "sc_svsg", c_sg)
                svout = _Series(ectx, scalar, s_v, SVd, "sc_svout", c_out)
                with scalar.Fori(0, repeat):
                    for j in range(R):
                        b, p, bx = j & 1, (j - 1) & 1, j % 4
                        bf, h = (j // 2) & 1, j & 1
                        # abs(j): yt[b] = |x|
                        xd[bx].wait(16 * (j // 4) + 16)
                        od[b].wait(16 * ((j + b) // 2))
                        nc.scalar.activation(
                            yt[b][:], xt[bx][:], AF.Abs).then_inc(s_a, 1)
                        # sigmoid(j) on this half's groups
                        fd[bf].wait(16 * (j // 4) + 16)
                        svsg.wait(O * j + c_sg)
                        nc.scalar.activation(
                            sg[b][:], ftu[bf][:, h * cw:(h + 1) * cw],
                            AF.Sigmoid).then_inc(s_a, 1)
                        # output of chunk j-1 (slot 0 of iter 0: phantom,
                        # overwritten by the real last-chunk output)
                        svout.wait(O * j + c_out)
                        scalar.dma_start(
                            out[xs(j - 1)], yt[p][:]).then_inc(s_od[p], 16)
                    for s in xd + od + fd + [svsg, svout]:
                        s.end_iter()
                # epilogue: final chunk's output
                scalar.wait_ge(s_v, SV_SEED + SVd * repeat + (3 if pool else 4))
                scalar.dma_start(
                    out[xs(R - 1)], yt[(R - 1) & 1][:]).then_inc(
                    s_od[(R - 1) & 1], 16)

        @block.gpsimd
        def _(gpsimd):
            if not pool:
                return
            with ExitStack() as ectx:
                pv = _Series(ectx, gpsimd, s_v, SVd, "p_sv", SV_SEED + 2)
                pa = _Series(ectx, gpsimd, s_a, SAd, "p_sa", 5)
                with gpsimd.Fori(0, repeat):
                    for j in range(R):
                        b, p = j & 1, (j - 1) & 1
                        # F(p): *= sc after TSmax(p)
                        pv.wait(O * j + SV_SEED + 2)
                        nc.gpsimd.tensor_tensor(
                            g3(yt[p]), g3(yt[p]), bc(sc[p]),
                            Alu.mult).then_inc(s_p, 1)
                        # C(j): yt = |x| * u2 after abs(j) and recip(j)
                        pa.wait(2 * j + 5)
                        pv.wait(O * j + SV_SEED + 6)
                        nc.gpsimd.tensor_tensor(
                            g3(yt[b]), g3(yt[b]), bc(u2[b]),
                            Alu.mult).then_inc(s_p, 1)
                    pv.end_iter()
                    pa.end_iter()
                # epilogue: F of the final chunk
                gpsimd.wait_ge(s_v, SV_SEED + SVd * repeat + 2)
                lp = (R - 1) & 1
                nc.gpsimd.tensor_tensor(
                    g3(yt[lp]), g3(yt[lp]), bc(sc[lp]),
                    Alu.mult).then_inc(s_p, 1)

        @block.vector
        def _(vector):
            with ExitStack() as ectx:
                vector.sem_inc(s_v, SV_SEED)
                vc = _Series(ectx, vector, s_v, SVd, "v_chain", SV_SEED)
                va = _Series(ectx, vector, s_a, SAd, "v_sa", 6)
                sp = _Series(ectx, vector, s_p, SPd, "v_sp", 0)
                xd = [_Series(ectx, vector, s_xd[k], XDd, f"v_xd{k}", 16)
                      for k in range(4)]
                with vector.Fori(0, repeat):
                    for j in range(R):
                        b, p, bx = j & 1, (j - 1) & 1, j % 4
                        px = (j - 1) % 4
                        n0 = SV_SEED + O * j
                        # V7(p): exponent mask (1x odd view; after C(p))
                        if pool:
                            sp.wait(2 * j)
                        vc.wait(n0 + 0)
                        nc.vector.tensor_scalar(
                            odd3(yt[p][:].bitcast(I32)),
                            odd3(yt[p][:].bitcast(I32)),
                            EXP_MASK, None, Alu.bitwise_and).then_inc(s_v, 1)
                        # TSmax(p): QMIN clip (1x odd view)
                        vc.wait(n0 + 1)
                        nc.vector.tensor_scalar(
                            odd3(yt[p][:]), odd3(yt[p][:]), TWO_M7, None,
                            Alu.max).then_inc(s_v, 1)
                        nf = 2
                        if not pool:
                            # F(p) on vector
                            vc.wait(n0 + 2)
                            nc.vector.tensor_tensor(
                                g3(yt[p]), g3(yt[p]), bc(sc[p]),
                                Alu.mult).then_inc(s_v, 1)
                            nf = 3
                        # reduce(j)
                        xd[bx].wait(16 * (j // 4) + 16)
                        vc.wait(n0 + nf)
                        nc.vector.tensor_reduce(
                            mx[b][:], g3(xt[bx]), axis=mybir.AxisListType.X,
                            op=Alu.max, apply_absolute_value=True,
                        ).then_inc(s_v, 1)
                        # sc = sg * mx
                        va.wait(2 * j + 6)
                        vc.wait(n0 + nf + 1)
                        nc.vector.tensor_tensor(
                            sc[b][:], sg[b][:], mx[b][:], Alu.mult,
                        ).then_inc(s_v, 1)
                        # tb = sc/sqrt2 ; u2 = 1/tb (exact HW divide)
                        vc.wait(n0 + nf + 2)
                        nc.vector.tensor_scalar(
                            tb[b][:], sc[b][:], INV_SQRT2, None,
                            Alu.mult).then_inc(s_v, 1)
                        vc.wait(n0 + nf + 3)
                        nc.vector.reciprocal(
                            u2[b][:], tb[b][:]).then_inc(s_v, 1)
                        if not pool:
                            # C(j) on vector
                            vc.wait(n0 + nf + 4)
                            nc.vector.tensor_tensor(
                                g3(yt[b]), g3(yt[b]), bc(u2[b]),
                                Alu.mult).then_inc(s_v, 1)
                        # V9(p): sign-OR (needs Pool F(p) done)
                        if pool:
                            sp.wait(2 * j + 1)
                        vc.wait(n0 + O - 1)
                        stt_int(
                            nc, nc.vector,
                            yt[p][:].bitcast(I32), xt[px][:].bitcast(I32),
                            SIGN_BIT, yt[p][:].bitcast(I32),
                            Alu.bitwise_and, Alu.bitwise_or,
                        ).then_inc(s_v, 1)
                    vc.end_iter()
                    va.end_iter()
                    sp.end_iter()
                    for s in xd:
                        s.end_iter()
                # epilogue: tail ops of the final chunk
                lp, lpx = (R - 1) & 1, (R - 1) % 4
                base = SV_SEED + SVd * repeat
                if pool:
                    vector.wait_ge(s_p, SPd * repeat)
                vector.wait_ge(s_v, base)
                nc.vector.tensor_scalar(
                    odd3(yt[lp][:].bitcast(I32)), odd3(yt[lp][:].bitcast(I32)),
                    EXP_MASK, None, Alu.bitwise_and).then_inc(s_v, 1)
                vector.wait_ge(s_v, base + 1)
                nc.vector.tensor_scalar(
                    odd3(yt[lp][:]), odd3(yt[lp][:]), TWO_M7, None,
                    Alu.max).then_inc(s_v, 1)
                if pool:
                    vector.wait_ge(s_p, SPd * repeat + 1)
                else:
                    vector.wait_ge(s_v, base + 2)
                    nc.vector.tensor_tensor(
                        g3(yt[lp]), g3(yt[lp]), bc(sc[lp]),
                        Alu.mult).then_inc(s_v, 1)
                vector.wait_ge(s_v, base + 2 + (0 if pool else 1))
                stt_int(
                    nc, nc.vector,
                    yt[lp][:].bitcast(I32), xt[lpx][:].bitcast(I32),
                    SIGN_BIT, yt[lp][:].bitcast(I32),
                    Alu.bitwise_and, Alu.bitwise_or).then_inc(s_v, 1)

    return nc


def build_loop_v3(sr=SR, d2=D2, repeat=1):
    """Fast path v3: Pool runs only C = x*u2 (signed; mask drops sign later).

    Half-width chunks, 4-deep x buffers, skewed slots.  Per slot j:
      DVE : V1 reduce(j); V2 sc=sg*mx; V3 tb=sc/sqrt2; V4 u2=recip(tb);
            V5 mask(j-1) @2x (Pool provably idle: C(j) is gated on this op);
            V6 STT (max 2^-7)*sc (j-1); V7 STT sign-OR (j-1)
      Pool: C(j): yt[b] = xt * u2 (bcast), gated on V5(j-1); owns yt WAR
      ACT : sigmoid(j); out-DMA of chunk j-1
      sync: x half-chunk in; fu row-block in (even slots)
    """
    d2g = d2 // GROUP
    assert d2g % 2 == 0 and sr % 256 == 0
    cw = d2g // 2
    ce = cw * GROUP
    R = (sr // 128) * 2
    assert R % 4 == 0
    O = 7
    SVd, SAd, SPd = O * R, R, R
    XDd, ODd, FDd = 4 * R, 8 * R, 4 * R
    SV_SEED, SA_SEED = 16, 2
    INV_SQRT2 = float(1.0 / np.sqrt(2.0))

    nc = bass.Bass()
    x = nc.dram_tensor("x", [sr, d2], F32, kind="ExternalInput")
    fu = nc.dram_tensor("fu", [sr, d2g], F32, kind="ExternalInput")
    out = nc.dram_tensor("out", [sr, d2], F32, kind="ExternalOutput")

    def xs(j):
        r, h = (j % R) // 2, (j % R) & 1
        return slice(r * 128, (r + 1) * 128), slice(h * ce, (h + 1) * ce)

    def g3(tile):
        return tile[:].rearrange("p (g e) -> p g e", g=cw)

    def bc(small):
        return small[:].unsqueeze(2).broadcast_to((128, cw, GROUP))

    with ExitStack() as ctx:
        def sbn(name, shape, n, dt=F32):
            return [ctx.enter_context(nc.sbuf_tensor(f"{name}{k}", shape, dt))
                    for k in range(n)]
        xt = sbn("xt", [128, ce], 4)
        yt = sbn("yt", [128, ce], 2)
        ftu = sbn("ftu", [128, d2g], 2)
        sg = sbn("sg", [128, cw], 2)
        mx = sbn("mx", [128, cw], 2)
        sc = sbn("sc", [128, cw], 2)
        tb = sbn("tb", [128, cw], 2)
        u2 = sbn("u2", [128, cw], 2)
        s_xd = [ctx.enter_context(nc.semaphore(f"s_xd{k}")) for k in range(4)]
        s_fd = [ctx.enter_context(nc.semaphore(f"s_fd{k}")) for k in range(2)]
        s_od = [ctx.enter_context(nc.semaphore(f"s_od{k}")) for k in range(2)]
        s_v = ctx.enter_context(nc.semaphore("s_v"))
        s_a = ctx.enter_context(nc.semaphore("s_a"))
        s_p = ctx.enter_context(nc.semaphore("s_p"))
        _emit_sem_init(nc, s_xd + s_fd + s_od + [s_v, s_a, s_p])
        block = ctx.enter_context(nc.Block())

        @block.sync
        def _(sync):
            with ExitStack() as ectx:
                sv = _Series(ectx, sync, s_v, SVd, "sy_sv", 2)
                sa = _Series(ectx, sync, s_a, SAd, "sy_sa", 0)
                with sync.Fori(0, repeat):
                    for j in range(R):
                        bx = j % 4
                        # xt[bx] free once V7sign(chunk j-4) read it
                        sv.wait(O * j + 2)
                        sync.dma_start(xt[bx][:], x[xs(j)]).then_inc(
                            s_xd[bx], 16)
                        if j % 2 == 0:
                            bf = (j // 2) & 1
                            sa.wait(j)
                            rs = slice((j // 2) * 128, (j // 2 + 1) * 128)
                            sync.dma_start(ftu[bf][:], fu[rs, :]).then_inc(
                                s_fd[bf], 16)
                    sv.end_iter()
                    sa.end_iter()

        @block.scalar
        def _(scalar):
            with ExitStack() as ectx:
                scalar.sem_inc(s_a, SA_SEED)
                fd = [_Series(ectx, scalar, s_fd[k], FDd, f"sc_fd{k}", 16)
                      for k in range(2)]
                svsg = _Series(ectx, scalar, s_v, SVd, "sc_svsg", 4)
                svout = _Series(ectx, scalar, s_v, SVd, "sc_svout", 23)
                with scalar.Fori(0, repeat):
                    for j in range(R):
                        b, p = j & 1, (j - 1) & 1
                        bf, h = (j // 2) & 1, j & 1
                        fd[bf].wait(16 * (j // 4) + 16)
                        svsg.wait(O * j + 4)
                        nc.scalar.activation(
                            sg[b][:], ftu[bf][:, h * cw:(h + 1) * cw],
                            AF.Sigmoid).then_inc(s_a, 1)
                        svout.wait(O * j + 23)
                        scalar.dma_start(
                            out[xs(j - 1)], yt[p][:]).then_inc(s_od[p], 16)
                    for s in fd + [svsg, svout]:
                        s.end_iter()
                scalar.wait_ge(s_v, SV_SEED + SVd * repeat + 3)
                scalar.dma_start(
                    out[xs(R - 1)], yt[(R - 1) & 1][:]).then_inc(
                    s_od[(R - 1) & 1], 16)

        @block.gpsimd
        def _(gpsimd):
            with ExitStack() as ectx:
                pv = _Series(ectx, gpsimd, s_v, SVd, "p_sv", 21)
                xd = [_Series(ectx, gpsimd, s_xd[k], XDd, f"p_xd{k}", 16)
                      for k in range(4)]
                od = [_Series(ectx, gpsimd, s_od[k], ODd, f"p_od{k}", 16 * k)
                      for k in range(2)]
                with gpsimd.Fori(0, repeat):
                    for j in range(R):
                        b, bx = j & 1, j % 4
                        # C(j): gated on V5 mask(j-1) so the 2x op never
                        # overlaps Pool (shared SBUF port pair)
                        xd[bx].wait(16 * (j // 4) + 16)
                        od[b].wait(16 * ((j + b) // 2))
                        pv.wait(O * j + 21)
                        nc.gpsimd.tensor_tensor(
                            g3(yt[b]), g3(xt[bx]), bc(u2[b]),
                            Alu.mult).then_inc(s_p, 1)
                    pv.end_iter()
                    for s in xd + od:
                        s.end_iter()

        @block.vector
        def _(vector):
            with ExitStack() as ectx:
                vector.sem_inc(s_v, SV_SEED)
                vc = _Series(ectx, vector, s_v, SVd, "v_chain", SV_SEED)
                va = _Series(ectx, vector, s_a, SAd, "v_sa", 3)
                sp = _Series(ectx, vector, s_p, SPd, "v_sp", 0)
                xd = [_Series(ectx, vector, s_xd[k], XDd, f"v_xd{k}", 16)
                      for k in range(4)]

                def tail(j, n0, wait_abs=None):
                    p, px = (j - 1) & 1, (j - 1) % 4
                    if wait_abs is None:
                        sp.wait(j)
                        vc.wait(n0 + 4)
                    else:
                        vector.wait_ge(s_p, SPd * repeat)
                        vector.wait_ge(s_v, wait_abs)
                    nc.vector.tensor_scalar(
                        yt[p][:].bitcast(I32), yt[p][:].bitcast(I32),
                        EXP_MASK, None, Alu.bitwise_and).then_inc(s_v, 1)
                    if wait_abs is None:
                        vc.wait(n0 + 5)
                    else:
                        vector.wait_ge(s_v, wait_abs + 1)
                    nc.vector.scalar_tensor_tensor(
                        g3(yt[p]), g3(yt[p]), TWO_M7, bc(sc[p]),
                        Alu.max, Alu.mult).then_inc(s_v, 1)
                    if wait_abs is None:
                        vc.wait(n0 + 6)
                    else:
                        vector.wait_ge(s_v, wait_abs + 2)
                    stt_int(
                        nc, nc.vector,
                        yt[p][:].bitcast(I32), xt[px][:].bitcast(I32),
                        SIGN_BIT, yt[p][:].bitcast(I32),
                        Alu.bitwise_and, Alu.bitwise_or).then_inc(s_v, 1)

                with vector.Fori(0, repeat):
                    for j in range(R):
                        b, bx = j & 1, j % 4
                        n0 = SV_SEED + O * j
                        xd[bx].wait(16 * (j // 4) + 16)
                        vc.wait(n0 + 0)
                        nc.vector.tensor_reduce(
                            mx[b][:], g3(xt[bx]), axis=mybir.AxisListType.X,
                            op=Alu.max, apply_absolute_value=True,
                        ).then_inc(s_v, 1)
                        va.wait(j + 3)
                        vc.wait(n0 + 1)
                        nc.vector.tensor_tensor(
                            sc[b][:], sg[b][:], mx[b][:], Alu.mult,
                        ).then_inc(s_v, 1)
                        vc.wait(n0 + 2)
                        nc.vector.tensor_scalar(
                            tb[b][:], sc[b][:], INV_SQRT2, None,
                            Alu.mult).then_inc(s_v, 1)
                        vc.wait(n0 + 3)
                        nc.vector.reciprocal(
                            u2[b][:], tb[b][:]).then_inc(s_v, 1)
                        tail(j, n0)
                    vc.end_iter()
                    va.end_iter()
                    sp.end_iter()
                    for s in xd:
                        s.end_iter()
                tail(R, None, wait_abs=SV_SEED + SVd * repeat)

    return nc


def build_shard_kernel(sr=SR, d2=D2, cg=86, equal=True, repeat=1,
                       sign_act=False, skip_clip=False):
    """General one-core program (unrolled, used off the fast path)."""
    d2g = d2 // GROUP
    assert d2g % cg == 0 and sr % 128 == 0
    hpb = d2g // cg
    rb = sr // 128
    nchunk = rb * hpb * repeat
    ce = cg * GROUP

    nc = bass.Bass()
    x = nc.dram_tensor("x", [sr, d2], F32, kind="ExternalInput")
    fu = nc.dram_tensor("fu", [sr, d2g], F32, kind="ExternalInput")
    fl = nc.dram_tensor("fl", [sr, d2g], F32, kind="ExternalInput")
    out = nc.dram_tensor("out", [sr, d2], F32, kind="ExternalOutput")

    def chunk_slices(i):
        r, h = divmod(i % (rb * hpb), hpb)
        rs = slice(r * 128, (r + 1) * 128)
        return (
            (rs, slice(h * ce, (h + 1) * ce)),
            (rs, slice(h * cg, (h + 1) * cg)),
        )

    with ExitStack() as ctx:
        def sb2(name, shape, dt=F32):
            return [
                ctx.enter_context(nc.sbuf_tensor(f"{name}{k}", shape, dt))
                for k in range(2)
            ]
        xt = sb2("xt", [128, ce])
        yt = sb2("yt", [128, ce])
        st = sb2("st", [128, ce]) if sign_act else None
        ftu = sb2("ftu", [128, cg])
        ftl = sb2("ftl", [128, cg])
        sg = sb2("sg", [128, cg])
        sl = sb2("sl", [128, cg])
        mx = sb2("mx", [128, cg])
        mn = sb2("mn", [128, cg])
        sc = sb2("sc", [128, cg])
        u2 = sb2("u2", [128, cg])
        ta = sb2("ta", [128, cg])
        tb = sb2("tb", [128, cg])
        s_xd = [ctx.enter_context(nc.semaphore("s_xd0")),
                ctx.enter_context(nc.semaphore("s_xd1"))]
        s_fd = [ctx.enter_context(nc.semaphore("s_fd0")),
                ctx.enter_context(nc.semaphore("s_fd1"))]
        s_od = [ctx.enter_context(nc.semaphore("s_od0")),
                ctx.enter_context(nc.semaphore("s_od1"))]
        s_v = ctx.enter_context(nc.semaphore("s_v"))
        s_a = ctx.enter_context(nc.semaphore("s_a"))
        block = ctx.enter_context(nc.Block())

        # ACT schedule per chunk: [sigmoid_u (, sigmoid_l), recip (, sign)]
        apc = (1 if equal else 2) + 1 + (1 if sign_act else 0)
        def a_sig_mark(i):      # sigmoids of chunk i done
            return i * apc + (1 if equal else 2)
        def a_recip_mark(i):    # + reciprocal seed done
            return i * apc + (2 if equal else 3)
        def a_done_mark(i):
            return (i + 1) * apc
        fpc = 1 if equal else 2

        v_done = {}
        v_sc_mark = {}
        v_marks = {"cnt": 0}

        @block.vector
        def _(vector):
            def V(make):
                vector.wait_ge(s_v, v_marks["cnt"])
                make().then_inc(s_v, 1)
                v_marks["cnt"] += 1

            for i in range(nchunk):
                b = i & 1
                x3 = xt[b][:].rearrange("p (g e) -> p g e", g=cg)
                y3 = yt[b][:].rearrange("p (g e) -> p g e", g=cg)
                vector.wait_ge(s_xd[b], 16 * (i // 2 + 1))
                V(lambda: nc.vector.tensor_reduce(
                    mx[b][:], x3, axis=mybir.AxisListType.X, op=Alu.max,
                    apply_absolute_value=equal))
                if not equal:
                    V(lambda: nc.vector.tensor_reduce(
                        mn[b][:], x3, axis=mybir.AxisListType.X, op=Alu.min))
                vector.wait_ge(s_a, a_sig_mark(i))
                if equal:
                    V(lambda: nc.vector.tensor_tensor(
                        sc[b][:], sg[b][:], mx[b][:], Alu.mult))
                else:
                    V(lambda: nc.vector.tensor_tensor(
                        ta[b][:], sg[b][:], mx[b][:], Alu.mult))
                    V(lambda: nc.vector.tensor_tensor(
                        tb[b][:], sl[b][:], mn[b][:], Alu.mult))
                    V(lambda: nc.vector.tensor_scalar(
                        ta[b][:].bitcast(I32), ta[b][:].bitcast(I32), SIGNLESS, None,
                        Alu.bitwise_and))
                    V(lambda: nc.vector.tensor_scalar(
                        tb[b][:].bitcast(I32), tb[b][:].bitcast(I32), SIGNLESS, None,
                        Alu.bitwise_and))
                    V(lambda: nc.vector.tensor_tensor(
                        sc[b][:], ta[b][:], tb[b][:], Alu.max))
                if not skip_clip:
                    V(lambda: nc.vector.tensor_scalar(
                        sc[b][:], sc[b][:], CLIPMIN, CLIPMAX, Alu.max, Alu.min))
                v_sc_mark[i] = v_marks["cnt"]
                # one NR step on the ACT reciprocal seed, sqrt2 folded in:
                # u2 = r * (2 - sc*r) * sqrt2
                vector.wait_ge(s_a, a_recip_mark(i))
                V(lambda: nc.vector.tensor_tensor(
                    tb[b][:], sc[b][:], ta[b][:], Alu.mult))
                V(lambda: nc.vector.tensor_scalar(
                    tb[b][:], tb[b][:], 2.0, -SQRT2, Alu.subtract, Alu.mult))
                V(lambda: nc.vector.tensor_tensor(
                    u2[b][:], ta[b][:], tb[b][:], Alu.mult))
                # elementwise stage
                if i >= 2:
                    vector.wait_ge(s_od[b], 16 * (i // 2))
                u2b = u2[b][:].unsqueeze(2).broadcast_to((128, cg, GROUP))
                scb = sc[b][:].unsqueeze(2).broadcast_to((128, cg, GROUP))
                V(lambda: nc.vector.tensor_tensor(y3, x3, u2b, Alu.mult))
                V(lambda: nc.vector.tensor_scalar(
                    yt[b][:], yt[b][:], 1.0, -1.0, Alu.min, Alu.max))
                V(lambda: nc.vector.tensor_scalar(
                    yt[b][:].bitcast(I32), yt[b][:].bitcast(I32), EXP_MASK, None,
                    Alu.bitwise_and))
                V(lambda: nc.vector.scalar_tensor_tensor(
                    y3, y3, TWO_M7, scb, Alu.max, Alu.mult))
                if sign_act:
                    vector.wait_ge(s_a, a_done_mark(i))
                    V(lambda: nc.vector.tensor_tensor(
                        yt[b][:], yt[b][:], st[b][:], Alu.mult))
                else:
                    V(lambda: nc.vector.tensor_scalar(
                        xt[b][:].bitcast(I32), xt[b][:].bitcast(I32),
                        SIGN_BIT, ONE_BITS, Alu.bitwise_and, Alu.bitwise_or))
                    V(lambda: nc.vector.tensor_tensor(
                        yt[b][:], yt[b][:], xt[b][:], Alu.mult))
                v_done[i] = v_marks["cnt"]

        @block.scalar
        def _(scalar):
            acnt = 0
            def A(make):
                nonlocal acnt
                scalar.wait_ge(s_a, acnt)
                make().then_inc(s_a, 1)
                acnt += 1

            for i in range(nchunk):
                b = i & 1
                if i >= 2:
                    scalar.wait_ge(s_v, v_done[i - 2])
                scalar.wait_ge(s_fd[b], 16 * fpc * (i // 2 + 1))
                A(lambda: nc.scalar.activation(sg[b][:], ftu[b][:], AF.Sigmoid))
                if not equal:
                    A(lambda: nc.scalar.activation(sl[b][:], ftl[b][:], AF.Sigmoid))
                scalar.wait_ge(s_v, v_sc_mark[i])
                A(lambda: act_reciprocal(nc, ta[b][:], sc[b][:]))
                if sign_act:
                    scalar.wait_ge(s_xd[b], 16 * (i // 2 + 1))
                    A(lambda: nc.scalar.activation(st[b][:], xt[b][:], AF.Sign))
                assert acnt == a_done_mark(i)

        @block.sync
        def _(sync):
            for i in range(nchunk):
                b = i & 1
                (xs_r, xs_c), (fs_r, fs_c) = chunk_slices(i)
                if i >= 2:
                    sync.wait_ge(s_v, v_done[i - 2])
                    sync.wait_ge(s_a, a_done_mark(i - 2))
                sync.dma_start(xt[b][:], x[xs_r, xs_c]).then_inc(s_xd[b], 16)
                sync.dma_start(ftu[b][:], fu[fs_r, fs_c]).then_inc(s_fd[b], 16)
                if not equal:
                    sync.dma_start(ftl[b][:], fl[fs_r, fs_c]).then_inc(s_fd[b], 16)
                if i >= 1:
                    (ps_r, ps_c), _ = chunk_slices(i - 1)
                    sync.wait_ge(s_v, v_done[i - 1])
                    sync.dma_start(out[ps_r, ps_c], yt[(i - 1) & 1][:]).then_inc(
                        s_od[(i - 1) & 1], 16)
            (ps_r, ps_c), _ = chunk_slices(nchunk - 1)
            sync.wait_ge(s_v, v_done[nchunk - 1])
            sync.dma_start(out[ps_r, ps_c], yt[(nchunk - 1) & 1][:]).then_inc(
                s_od[(nchunk - 1) & 1], 16)

    return nc


# "v1" = all-DVE hardware loop (default; ~230us/pass, DVE-bound).  "v2" =
# DVE+Pool+ACT op split — measured SLOWER (316us/pass): GpSimd TensorTensor
# runs at ~2.66 ns/elem (not the cost model's 1.98) so Pool with both big
# multiplies is the bottleneck; a rebalanced split (Pool takes only y=x*u2)
# projects ~170us/pass but is unvalidated.  Both builders self-initialize
# their semaphores at entry (_emit_sem_init) — executions are NOT guaranteed
# fresh sems; stale sems masqueraded as a Pool race before.
FAST_BUILDER = "v1"


@functools.lru_cache(maxsize=8)
def _cached_kernel(sr, d2, cg, equal, repeat=1, sign_act=False, skip_clip=False):
    d2g = d2 // GROUP
    if (equal and skip_clip and not sign_act and cg == d2g
            and sr % 256 == 0 and FAST_BUILDER != "legacy"):
        if (FAST_BUILDER in ("v2", "v3") and d2g % 2 == 0
                and ((sr // 128) * 2) % 4 == 0):
            if FAST_BUILDER == "v3":
                return build_loop_v3(sr, d2, repeat)
            return build_loop_v2(sr, d2, repeat)
        return build_loop_kernel(sr, d2, cg, repeat)
    return build_shard_kernel(sr, d2, cg, equal, repeat, sign_act, skip_clip)


def _pick_config(x, fu, fl):
    equal = bool(np.array_equal(fu, fl))
    has_zero = bool((x == 0.0).any())
    d2g = x.shape[1] // GROUP
    if has_zero:
        # ACT-Sign variant needs the third big tile; halve the chunk width.
        cg = d2g // 2 if d2g % 2 == 0 else d2g
        return dict(cg=cg, equal=equal, sign_act=True, skip_clip=False)
    ga = np.abs(x).reshape(-1, GROUP).max(axis=1)
    sig_lo = 1.0 / (1.0 + np.exp(-float(min(fu.min(), fl.min()))))
    sig_hi = 1.0 / (1.0 + np.exp(-float(max(fu.max(), fl.max()))))
    # skip_clip means every redundant clamp is provably dead: the scale clip
    # (group scales strictly inside (1e-5, 1e4)) and the QMAX clip
    # (sigmoid > 1/sqrt2 so |x|*sqrt2/scale < 2).
    skip_clip = bool(ga.min() * sig_lo > 2e-5 and ga.max() * sig_hi < 5e3
                     and sig_lo > 0.70711)
    return dict(cg=d2g, equal=equal, sign_act=False, skip_clip=skip_clip)


def run_sharded(x, upbound_factor, lowbound_factor, repeat=1):
    d1, d2 = x.shape
    sr = d1 // N_CORES
    d2g = d2 // GROUP
    fu = np.ascontiguousarray(upbound_factor.reshape(d1, d2g), dtype=np.float32)
    fl = np.ascontiguousarray(lowbound_factor.reshape(d1, d2g), dtype=np.float32)
    cfg = _pick_config(x, fu, fl)
    nc = _cached_kernel(sr, d2, cfg["cg"], cfg["equal"], repeat,
                        cfg["sign_act"], cfg["skip_clip"])
    in_maps = []
    for c in range(N_CORES):
        rs = slice(c * sr, (c + 1) * sr)
        in_maps.append({
            "x": np.ascontiguousarray(x[rs], dtype=np.float32),
            "fu": fu[rs],
            "fl": fl[rs],
        })
    res = run_bass_kernel_spmd(nc, in_maps, list(range(N_CORES)))
    full = np.concatenate([res.results[c]["out"] for c in range(N_CORES)], axis=0)
    return full, res


def kernel(x, upbound_factor, lowbound_factor):
    x = np.asarray(x, dtype=np.float32)
    upbound_factor = np.asarray(upbound_factor, dtype=np.float32)
    lowbound_factor = np.asarray(lowbound_factor, dtype=np.float32)
    full, _ = run_sharded(x, upbound_factor, lowbound_factor)
    return full
